# revision 23
# baseline (speedup 1.0000x reference)
"""Trainium2 Bass kernel for nn_Block_9199819948105 (dense_cnn) — v3.

Per core (2 of 16 batches, data-parallel over 8 cores):
  conv1 (stride-2 7^3) as z-Toeplitz banded matmuls with ky tap-PAIR packing;
  psum cols ordered (i, uhalf, zor, u4); the psum->SBUF copies write QUADRANT
  (x/y parity) planes so conv2's tap views are contiguous-inner; conv1->conv2
  regather is per-(i,uhalf) contiguous SBUF->SBUF DMAs into u-half-split
  v tiles (lo: u0-3, hi: u4-7) so batch-1 gathers overlap conv2-batch-0;
  tensor product on DVE; conv2 via rank-3 basis z-Toeplitz matmuls
  (quadrant-contiguous rhs); 1x1 mix drains straight into a folded
  yfin[128,1715] SBUF tile (no DRAM round trip, no accumulator stalls);
  stats via one reduce+square pass at the end; BN all-reduced across the
  8 cores; fused scale/shift+relu in place on yfin.
"""
import sys
import numpy as np

sys.path.insert(0, '/opt/trn_rl_repo')

import ml_dtypes

BF16 = ml_dtypes.bfloat16

# ---------------- problem constants ----------------
N_CORES = 8
B, CIN, D0 = 16, 4, 64
VEC, SOUT, K, NB = 8, 16, 7, 3
D1 = 34
D2 = 19
XY2 = D2 * D2            # 361
NV2 = D2 * XY2           # 6859
EPS = 1e-5
BB = B // N_CORES        # 2
NTOT = B * NV2
FP1 = 44 * 44            # per-u plane, stored as 4 quadrants of 22x22
QP = 22 * 22             # 484 quadrant plane
SQF = 2 * 37 * 38        # conv1 input quarter-planes (xp, 37 xq, 38 yh)

ZBLK = [(0, 0, 10, 5), (1, 5, 20, 5), (2, 15, 30, 5), (3, 25, 40, 5),
        (4, 35, 50, 5), (5, 45, 60, 5), (6, 55, 64, 4)]
VAR = [(10, 5, 5), (15, 5, 0), (9, 4, 0)]   # (nzr, Zo, kzoff)
KY0 = [0, 2, 4, 6]
XCH = [(0, 15), (15, 30), (30, 34)]          # conv1 x chunks -> free 510/510/136
PADN = 8 * 1715          # 13720: BB*NV2 (13718) padded to 8 chunks
CV = PADN // 8           # 1715


# ---------------- host-side weight prep ----------------

def _build_w1t(W1, basis1):
    """conv1 weights; psum col order (i, uhalf, zor, u4)."""
    K1 = np.einsum('uvb,bixyz->uivxyz', W1, basis1[:, :, 0]).reshape(24, 4, K, K, K)
    out = np.zeros((3, 28, 120, 120), np.float32)
    for vi, (nzr, Zo, kzoff) in enumerate(VAR):
        zr = np.arange(nzr)[:, None]
        zor = np.arange(Zo)[None, :]
        kz = zr - 2 * zor + kzoff
        mask = (kz >= 0) & (kz < 7)
        kzc = np.clip(kz, 0, 6)
        for kx in range(7):
            for yg in range(4):
                g = kx * 4 + yg
                nsh = 2 if yg < 3 else 1
                for s2 in range(nsh):
                    ky = KY0[yg] + s2
                    vals = K1[:, :, kx, ky, :][:, :, kzc] * mask      # [24,4,nzr,Zo]
                    m = vals.transpose(2, 1, 0, 3)                    # [zr,ci,co,zor]
                    m = m.reshape(nzr, 4, 8, 3, Zo)                   # co=(u,i)
                    # cols: (i, uh, zor, u4)
                    m = m.reshape(nzr, 4, 2, 4, 3, Zo)                # (u -> uh,u4)
                    m = m.transpose(0, 1, 4, 2, 5, 3)                 # [zr,ci,i,uh,zor,u4]
                    m = m.reshape(4 * nzr, 3 * Zo * 8)
                    out[vi, g, s2 * 4 * nzr:(s2 + 1) * 4 * nzr, :3 * Zo * 8] = m
    # device layout: [row(120), (vi*28+g)*120 + col]
    return np.ascontiguousarray(
        out.transpose(2, 0, 1, 3).reshape(120, 3 * 28 * 120)).astype(BF16)


def _build_w2t(basis2a, basis2b):
    zeta = np.arange(D1)[:, None]
    zo2 = np.arange(D2)[None, :]
    kz = zeta - 2 * zo2 + 5
    mask = (kz >= 0) & (kz < 7)
    kzc = np.clip(kz, 0, 6)
    W = np.zeros((3, 49, 128, 64), np.float32)
    for fam in range(3):
        for t in range(49):
            kx, ky = divmod(t, 7)
            for i in range(3):
                if fam == 0:
                    prof = basis2a[:, 0, i, kx, ky, :]
                elif fam == 1:
                    prof = basis2b[:, 0, i * 3 + i, kx, ky, :]
                else:
                    p = (i + 1) % 3
                    prof = basis2b[:, 0, i * 3 + p, kx, ky, :] + \
                        basis2b[:, 0, p * 3 + i, kx, ky, :]
                for b in range(NB):
                    vals = prof[b][kzc] * mask
                    W[fam, t, i * D1:(i + 1) * D1, b * D2:(b + 1) * D2] = vals
    W = W.reshape(147, 128, 64)
    return np.ascontiguousarray(
        W.transpose(1, 0, 2).reshape(128, 147 * 64)).astype(BF16)


def _build_wmix(W2a, W2b):
    M = np.zeros((48, 16), np.float32)
    for famM, W2 in [(0, W2a), (1, W2b)]:
        for u in range(VEC):
            for b in range(NB):
                M[famM * 24 + u * 3 + b, :] = W2[:, u, b]
    return M.astype(BF16)


def _prep_s(s_core):
    """[BB,4,64,64,64] -> 7 arrays [BB, 2*4*win, 2*37*38] bf16.
    row = s2*(4*win) + zr*4 + ci ; s2 rows hold the even(0)/odd(1) y
    half-plane, with free split by x parity: free = xp*37*38 + xq*38 + yh.
    Axes: x=D(pad 74), y=H(pad 76), z=W."""
    sp = np.zeros((BB, 4, 74, 76, 64), np.float32)
    sp[:, :, 5:69, 5:69, :] = s_core
    out = []
    for zb, wlo, whi, Zo in ZBLK:
        win = whi - wlo
        sl = sp[:, :, :, :, wlo:whi]                            # [BB,4,74,76,win]
        rows = []
        for s2 in range(2):
            a = sl[:, :, :, s2::2, :][:, :, :, :38, :]
            if a.shape[3] < 38:
                a = np.pad(a, ((0, 0), (0, 0), (0, 0),
                               (0, 38 - a.shape[3]), (0, 0)))
            # x-parity split: [BB,4,xp(2),37,38,win]
            a = np.stack([a[:, :, 0::2], a[:, :, 1::2]], axis=2)
            a = a.transpose(0, 5, 1, 2, 3, 4)                   # [BB,win,4,2,37,38]
            rows.append(a.reshape(BB, win * 4, SQF))
        out.append(np.ascontiguousarray(
            np.concatenate(rows, axis=1)).astype(BF16))
    return out


# ---------------- device program ----------------

def _build_program(n_cores):
    import concourse.bacc as bacc
    import concourse.mybir as mybir
    import concourse.tile as tile

    F32 = mybir.dt.float32
    BF = mybir.dt.bfloat16
    AF = mybir.ActivationFunctionType

    nc = bacc.Bacc("TRN2", target_bir_lowering=False, debug=False,
                   enable_asserts=True, num_devices=n_cores,
                   num_swdge_queues=4)

    sq_d = [nc.dram_tensor(f"sq{zb}", [BB, 8 * (whi - wlo), SQF], BF,
                           kind="ExternalInput").ap()
            for zb, wlo, whi, Zo in ZBLK]
    w1t_d = nc.dram_tensor("w1t", [120, 3 * 28 * 120], BF, kind="ExternalInput").ap()
    w2t_d = nc.dram_tensor("w2t", [128, 147 * 64], BF, kind="ExternalInput").ap()
    wmix_d = nc.dram_tensor("wmix", [48, 16], BF, kind="ExternalInput").ap()
    gvec_d = nc.dram_tensor("gvec", [16, 2], F32, kind="ExternalInput").ap()
    yout_d = nc.dram_tensor("yout", [16, PADN], F32, kind="ExternalOutput").ap()

    with tile.TileContext(nc) as tc:
        with tc.tile_pool(name="wpool", bufs=1) as wpool, \
             tc.tile_pool(name="big", bufs=1) as big, \
             tc.tile_pool(name="sqp", bufs=5) as sqp, \
             tc.tile_pool(name="tp", bufs=2) as tpp, \
             tc.tile_pool(name="d2s", bufs=2) as d2sp, \
             tc.tile_pool(name="bn", bufs=1) as bnp, \
             tc.tile_pool(name="ps", bufs=2, space="PSUM") as psp, \
             tc.tile_pool(name="dram", bufs=1, space="DRAM") as dramp:

            w1t = wpool.tile([120, 3 * 28 * 120], BF, tag="w1t")
            w2t = wpool.tile([128, 147 * 64], BF, tag="w2t")
            wmix = wpool.tile([48, 16], BF, tag="wmix")
            gvec = wpool.tile([16, 2], F32, tag="gvec")

            # u-half-split v tiles: [102, 4u, quadrant(2,2), 22, 22]
            vhalves = []
            for nm in ("vml", "vmh", "vpl", "vph"):
                t = big.tile([102, 4 * FP1], BF, tag=nm, name=nm)
                vhalves.append(t)
            vml, vmh, vpl, vph = vhalves
            # memsets split across engines (pads must be zero)
            nc.vector.memset(vml[:], 0.0)
            nc.vector.memset(vpl[:], 0.0)
            nc.gpsimd.memset(vmh[:], 0.0)
            nc.gpsimd.memset(vph[:], 0.0)

            m_in = big.tile([48, NV2], BF, tag="min")
            yfin = big.tile([128, CV], F32, tag="yfin")
            nc.vector.memset(yfin[:, CV - 2:CV], 0.0)   # pad tail
            sqscr = big.tile([128, 512], F32, tag="sqscr")  # square scratch

            vstgs = [big.tile([120, FP1], BF, tag=f"vstg{i}", name=f"vstg{i}")
                     for i in range(7)]
            for i, v in enumerate(vstgs):
                (nc.gpsimd if i % 2 == 0 else nc.vector).memset(v[:], 0.0)

            bn_in = dramp.tile([16, 2], F32, tag="bnin")
            bn_out = dramp.tile([16, 2], F32, tag="bnout")

            # views: [102, u4, xp, yp, 22, 22]
            def qview(t):
                return t[:].rearrange("p (u a b x y) -> p u a b x y",
                                      u=4, a=2, b=2, x=22)
            vmlq, vmhq, vplq, vphq = [qview(t) for t in vhalves]

            # initial loads; ring discipline:
            #  sync   = w1t slabs + lo-gathers (+ a few sq)
            #  scalar = sq zb0/2/4/6 + sq bb1 prefetch + m_in drains + mix
            #  gpsimd = sq zb1/3/5 + w2t + hi-gathers
            nc.sync.dma_start(w1t[:, 0:14 * 120], w1t_d[:, 0:14 * 120])
            nc.scalar.dma_start(w1t[:, 14 * 120:28 * 120],
                                w1t_d[:, 14 * 120:28 * 120])
            for vi0 in (1, 2):
                nc.sync.dma_start(w1t[:, vi0 * 28 * 120:(vi0 + 1) * 28 * 120],
                                  w1t_d[:, vi0 * 28 * 120:(vi0 + 1) * 28 * 120])
            sq_tiles = {}
            ld_rings = {0: nc.scalar, 1: nc.gpsimd, 2: nc.scalar, 3: nc.gpsimd,
                        4: nc.scalar, 5: nc.gpsimd, 6: nc.scalar}
            for zb, wlo, whi, Zo in ZBLK:
                win = whi - wlo
                sqt = sqp.tile([8 * win, SQF], BF, tag="sqz", name="sqt")
                ld_rings[zb].dma_start(sqt[:], sq_d[zb][0])
                sq_tiles[(0, zb)] = sqt
            nc.gpsimd.dma_start(w2t[:], w2t_d[:])
            nc.scalar.dma_start(wmix[:], wmix_d[:])
            nc.scalar.dma_start(gvec[:], gvec_d[:])
            rings = [nc.sync, nc.scalar, nc.gpsimd]

            def emit_conv1_zb(bb, zbi):
                zb, wlo, whi, Zo = ZBLK[zbi]
                win = whi - wlo
                vi = 0 if zb == 0 else (2 if zb == 6 else 1)
                ncols = 24 * Zo
                sqt = sq_tiles[(bb, zb)]
                sqv = sqt[0:8 * win, :].rearrange("p (e x y) -> p e x y",
                                                  e=2, y=38)
                vstg = vstgs[zbi]
                vsq = vstg[:].rearrange("p (a b x y) -> p a b x y",
                                        a=2, b=2, x=22)
                rows = 8 * win
                for cc, (clo, chi) in enumerate(XCH):
                    cx = chi - clo
                    pc = psp.tile([128, 512], F32, tag="pc", bufs=2)
                    for kx in range(7):
                        for yg in range(4):
                            g = kx * 4 + yg
                            lhs = w1t[0:rows, (vi * 28 + g) * 120:
                                      (vi * 28 + g) * 120 + ncols]
                            xq0 = clo + kx // 2
                            rhs = sqv[0:rows, kx % 2, xq0:xq0 + cx,
                                      yg:yg + 34]
                            nc.tensor.matmul(pc[0:ncols, 0:cx * 34], lhs, rhs,
                                             start=(g == 0), stop=(g == 27))
                    # psum -> vstg quadrant copies (4 per chunk)
                    pcv = pc[0:ncols, 0:cx * 34].rearrange(
                        "p (x y) -> p x y", y=34)
                    x0g = 5 + clo                    # global x of chunk col 0
                    for a in range(2):
                        dx0 = (a - x0g) % 2          # first dx with parity a
                        if dx0 >= cx:
                            continue
                        nxa = (cx - dx0 + 1) // 2
                        xq0d = (x0g + dx0 - a) // 2
                        for bpar in range(2):
                            dy0 = (bpar - 5) % 2     # first dy: y=5+dy par b
                            yq0d = (5 + dy0 - bpar) // 2
                            nc.vector.tensor_copy(
                                vsq[0:ncols, a, bpar,
                                    xq0d:xq0d + nxa, yq0d:yq0d + 17],
                                pcv[:, dx0::2, dy0::2])
                if bb == 0:
                    # prefetch next batch's sq for this zb (scalar ring)
                    sqt1 = sqp.tile([8 * win, SQF], BF, tag="sqz",
                                    name="sqt1")
                    nc.scalar.dma_start(sqt1[:], sq_d[zb][1])
                    sq_tiles[(1, zb)] = sqt1

            def emit_gather(zbi, uh):
                # vstg rows (i, uh, zor, u4) -> v half tiles.  lo halves on
                # sync, hi halves on gpsimd (separate rings so a WAR-blocked
                # hi gather never clogs the lo ring).  Emission point defines
                # program-order semantics: only emit a half's gather once all
                # prior readers of that half have been emitted.
                zb, wlo, whi, Zo = ZBLK[zbi]
                vstg = vstgs[zbi]
                zsp = [(0, Zo)] if zbi < 5 else [(0, 3), (3, Zo)]
                for i in range(3):
                    c = (i + 2) % 3
                    base = (i * 2 + uh) * 4 * Zo
                    vmq = vmlq if uh == 0 else vmhq
                    vpq = vplq if uh == 0 else vphq
                    ring = nc.sync if uh == 0 else nc.gpsimd
                    for z0, z1 in zsp:
                        src = vstg[base + z0 * 4: base + z1 * 4, :]
                        dsts = [vmq[i * D1 + 5 * zb + z0:
                                    i * D1 + 5 * zb + z1, :, :, :, :, :],
                                vpq[c * D1 + 5 * zb + z0:
                                    c * D1 + 5 * zb + z1, :, :, :, :, :]]
                        for dst in dsts:
                            ring.dma_start(dst, src)

            def emit_conv2_u(bb, u):
                vh = vhalves[0] if u < 4 else vhalves[1]
                vp = vhalves[2] if u < 4 else vhalves[3]
                u4 = u % 4
                vmu = vh[:].rearrange("p (u f) -> p u f", u=4)[:, u4, :]
                vpu = vp[:].rearrange("p (u f) -> p u f", u=4)[:, u4, :]
                t1u = tpp.tile([102, FP1], BF, tag="t1u")
                t2u = tpp.tile([102, FP1], BF, tag="t2u")
                nc.vector.tensor_mul(t1u[:], vmu, vmu)
                nc.vector.tensor_mul(t2u[:], vmu, vpu)
                pd2a = psp.tile([64, 512], F32, tag="pd2a", bufs=2)
                pd2b = psp.tile([64, 512], F32, tag="pd2b", bufs=2)
                pav = pd2a[0:57, 0:XY2].rearrange("p (x y) -> p x y", y=D2)
                pbv = pd2b[0:57, 0:XY2].rearrange("p (x y) -> p x y", y=D2)
                vmuq = vmu.rearrange("p (a b x y) -> p a b x y",
                                     a=2, b=2, x=22)
                t1q = t1u[:].rearrange("p (a b x y) -> p a b x y",
                                       a=2, b=2, x=22)
                t2q = t2u[:].rearrange("p (a b x y) -> p a b x y",
                                       a=2, b=2, x=22)
                for fam, rq, pv in ((0, vmuq, pav), (1, t1q, pbv),
                                    (2, t2q, pbv)):
                    for t in range(49):
                        kx, ky = divmod(t, 7)
                        rhs = rq[:, kx % 2, ky % 2,
                                 kx // 2:kx // 2 + 19,
                                 ky // 2:ky // 2 + 19]
                        lhs = w2t[0:102,
                                  (fam * 49 + t) * 64:(fam * 49 + t) * 64 + 57]
                        nc.tensor.matmul(pv[:, :, :], lhs, rhs,
                                         start=(t == 0 and fam != 2),
                                         stop=(t == 48 and fam != 1))
                for famM, psrc in ((0, pd2a), (1, pd2b)):
                    stg = d2sp.tile([57, XY2], BF, tag=f"stg{famM}")
                    nc.vector.tensor_copy(stg[:], psrc[0:57, 0:XY2])
                    # SBUF->SBUF: [57=(b,zo), 361] -> m_in [3 rows, 6859]
                    nc.scalar.dma_start(
                        m_in[famM * 24 + u * 3: famM * 24 + u * 3 + 3, :],
                        stg[:])

            def emit_mix(bb):
                nchunks = (NV2 + 511) // 512
                for ch in range(nchunks):
                    c0 = ch * 512
                    cn = min(512, NV2 - c0)
                    pm = psp.tile([16, 512], F32, tag="pm", bufs=2)
                    nc.tensor.matmul(pm[0:16, 0:cn], wmix[:], m_in[:, c0:c0 + cn],
                                     start=True, stop=True)
                    ymix = d2sp.tile([16, 512], F32, tag="ymix", bufs=3)
                    if ch % 2 == 0:
                        nc.vector.tensor_copy(ymix[0:16, 0:cn], pm[0:16, 0:cn])
                    else:
                        nc.scalar.copy(ymix[0:16, 0:cn], pm[0:16, 0:cn])
                    # fold into yfin[j*16+c, v]: global g = bb*NV2 + c0 + k
                    g0 = bb * NV2 + c0
                    g1 = g0 + cn
                    j0, v0 = divmod(g0, CV)
                    j1 = (g1 - 1) // CV
                    ring = (nc.gpsimd, nc.sync)[ch % 2]
                    if j0 == j1:
                        ring.dma_start(
                            yfin[j0 * 16:(j0 + 1) * 16, v0:v0 + cn],
                            ymix[0:16, 0:cn])
                    else:
                        n1 = CV - v0
                        ring.dma_start(
                            yfin[j0 * 16:(j0 + 1) * 16, v0:CV],
                            ymix[0:16, 0:n1])
                        ring.dma_start(
                            yfin[j1 * 16:(j1 + 1) * 16, 0:cn - n1],
                            ymix[0:16, n1:cn])

            # ---- software-pipelined schedule ----
            # bb0 conv1: compute + both gathers immediately (v tiles fresh)
            for zbi in range(7):
                emit_conv1_zb(0, zbi)
                emit_gather(zbi, 0)
                emit_gather(zbi, 1)
            # conv2-bb0 u0..3 read the lo halves; after u3 all of bb1's
            # conv1 runs (private vstg buffers), each z-block's lo gather
            # emitted immediately (only hi planes still being read by
            # u4..7).  Hi gathers wait until after u7 and run on gpsimd
            # under conv2-bb1's u0..3.
            for u in range(4):
                emit_conv2_u(0, u)
            for zbi in range(7):
                emit_conv1_zb(1, zbi)
                emit_gather(zbi, 0)
            for u in range(4, VEC):
                emit_conv2_u(0, u)
            for zbi in range(7):
                emit_gather(zbi, 1)
            emit_mix(0)
            for u in range(VEC):
                emit_conv2_u(1, u)
            emit_mix(1)

            # ---------------- stats + batchnorm all-reduce + finalize ------
            s128 = bnp.tile([128, 8], F32, tag="s128")
            nc.vector.reduce_sum(s128[:, 6:7], yfin[:], axis=mybir.AxisListType.X)
            SC4 = (CV + 3) // 4
            for c in range(4):
                v0 = c * SC4
                v1 = min(CV, v0 + SC4)
                nc.scalar.activation(sqscr[:, 0:v1 - v0], yfin[:, v0:v1],
                                     AF.Square, accum_out=s128[:, 1 + c:2 + c])
            nc.vector.reduce_sum(s128[:, 7:8], s128[:, 1:5],
                                 axis=mybir.AxisListType.X)
            # fold 128 -> 16: [128,2] -> [16, 16] (j-major pairs)
            fold = bnp.tile([16, 16], F32, tag="fold")
            for j in range(8):
                rings[j % 2].dma_start(fold[:, j * 2:(j + 1) * 2],
                                       s128[j * 16:(j + 1) * 16, 6:8])
            bnv = bnp.tile([16, 2], F32, tag="bnv")
            fv2 = fold[:].rearrange("p (j s) -> p s j", s=2)
            nc.vector.reduce_sum(bnv[:, 0:1], fv2[:, 0, :], axis=mybir.AxisListType.X)
            nc.vector.reduce_sum(bnv[:, 1:2], fv2[:, 1, :], axis=mybir.AxisListType.X)
            nc.sync.dma_start(bn_in[:], bnv[:])
            nc.gpsimd.collective_compute(
                "AllReduce", mybir.AluOpType.add,
                replica_groups=[list(range(n_cores))],
                ins=[bn_in[:].opt()], outs=[bn_out[:].opt()])
            bnr = bnp.tile([16, 2], F32, tag="bnr")
            nc.sync.dma_start(bnr[:], bn_out[:])
            w = bnp.tile([16, 8], F32, tag="bnw")
            invN = 1.0 / float(NTOT)
            nc.vector.tensor_scalar_mul(w[:, 0:1], bnr[:, 0:1], invN)   # mean
            nc.vector.tensor_scalar_mul(w[:, 1:2], bnr[:, 1:2], invN)   # E[x^2]
            nc.vector.tensor_mul(w[:, 2:3], w[:, 0:1], w[:, 0:1])       # mean^2
            nc.vector.tensor_sub(w[:, 3:4], w[:, 1:2], w[:, 2:3])       # var
            nc.vector.tensor_scalar_add(w[:, 4:5], w[:, 3:4], EPS)
            nc.vector.reciprocal(w[:, 5:6], w[:, 4:5])
            nc.scalar.sqrt(w[:, 6:7], w[:, 5:6])                        # rstd
            sc = bnp.tile([16, 2], F32, tag="bnsc")
            nc.vector.tensor_mul(sc[:, 0:1], gvec[:, 0:1], w[:, 6:7])   # scale
            nc.vector.tensor_mul(w[:, 7:8], w[:, 0:1], sc[:, 0:1])      # mean*sc
            nc.vector.tensor_sub(sc[:, 1:2], gvec[:, 1:2], w[:, 7:8])   # shift
            # broadcast to 128 partitions: 8 parallel DMAs across rings
            sc128 = bnp.tile([128, 2], F32, tag="sc128")
            for j in range(8):
                rings[j % 2].dma_start(sc128[j * 16:(j + 1) * 16, :], sc[:])
            # final: scale/shift + relu in place, write out (4 chunks,
            # store overlaps the next chunk's activation)
            yov = yout_d[:].rearrange("c (j v) -> j c v", j=8)
            NC4 = (CV + 3) // 4
            for k in range(4):
                v0 = k * NC4
                v1 = min(CV, v0 + NC4)
                nc.scalar.activation(yfin[:, v0:v1], yfin[:, v0:v1], AF.Relu,
                                     bias=sc128[:, 1:2], scale=sc128[:, 0:1])
                rings[k % 2].dma_start(yov[:, :, v0:v1], yfin[:, v0:v1])

    nc.compile()
    return nc


_CACHE = {}


def _get_program(n_cores):
    if n_cores not in _CACHE:
        _CACHE[n_cores] = _build_program(n_cores)
    return _CACHE[n_cores]


def _make_in_maps(inputs):
    s = np.asarray(inputs['s'], np.float32)
    w1t = _build_w1t(np.asarray(inputs['W1'], np.float32),
                     np.asarray(inputs['basis1'], np.float32))
    w2t = _build_w2t(np.asarray(inputs['basis2a'], np.float32),
                     np.asarray(inputs['basis2b'], np.float32))
    wmix = _build_wmix(np.asarray(inputs['W2a'], np.float32),
                       np.asarray(inputs['W2b'], np.float32))
    gvec = np.stack([np.asarray(inputs['gamma'], np.float32),
                     np.asarray(inputs['beta'], np.float32)
                     + np.asarray(inputs['bias'], np.float32)], axis=1)
    in_maps = []
    for c in range(N_CORES):
        sqs = _prep_s(s[BB * c: BB * (c + 1)])
        m = {f"sq{zb}": sqs[zb] for zb in range(7)}
        m.update({"w1t": w1t, "w2t": w2t, "wmix": wmix,
                  "gvec": np.ascontiguousarray(gvec)})
        in_maps.append(m)
    return in_maps


def _assemble(results):
    out = np.zeros((B, 16, D2, D2, D2), np.float32)
    for c in range(N_CORES):
        yo = results[c]["yout"]           # [16, 13720] (padded)
        for bb in range(BB):
            yb = yo[:, bb * NV2:(bb + 1) * NV2].reshape(16, D2, D2, D2)
            out[BB * c + bb] = yb.transpose(0, 2, 3, 1)  # (z,x,y)->(x,y,z)
    return out


def _run(inputs, trace=False, trace_kwargs=None):
    from concourse import bass_utils
    nc = _get_program(N_CORES)
    in_maps = _make_in_maps(inputs)
    res = bass_utils.run_bass_kernel_spmd(
        nc, in_maps, core_ids=list(range(N_CORES)), trace=trace,
        **(trace_kwargs or {}))
    return _assemble(res.results), res


def kernel(**inputs):
    out, _ = _run(inputs, trace=False)
    return out


# revision 24
# speedup vs baseline: 1.0273x; 1.0273x over previous
"""Trainium2 Bass kernel for nn_Block_9199819948105 (dense_cnn) — v3.

Per core (2 of 16 batches, data-parallel over 8 cores):
  conv1 (stride-2 7^3) as z-Toeplitz banded matmuls with ky tap-PAIR packing;
  psum cols ordered (i, uhalf, zor, u4); the psum->SBUF copies write QUADRANT
  (x/y parity) planes so conv2's tap views are contiguous-inner; conv1->conv2
  regather is per-(i,uhalf) contiguous SBUF->SBUF DMAs into u-half-split
  v tiles (lo: u0-3, hi: u4-7) so batch-1 gathers overlap conv2-batch-0;
  tensor product on DVE; conv2 via rank-3 basis z-Toeplitz matmuls
  (quadrant-contiguous rhs); 1x1 mix drains straight into a folded
  yfin[128,1715] SBUF tile (no DRAM round trip, no accumulator stalls);
  stats via one reduce+square pass at the end; BN all-reduced across the
  8 cores; fused scale/shift+relu in place on yfin.
"""
import sys
import numpy as np

sys.path.insert(0, '/opt/trn_rl_repo')

import ml_dtypes

BF16 = ml_dtypes.bfloat16

# ---------------- problem constants ----------------
N_CORES = 8
B, CIN, D0 = 16, 4, 64
VEC, SOUT, K, NB = 8, 16, 7, 3
D1 = 34
D2 = 19
XY2 = D2 * D2            # 361
NV2 = D2 * XY2           # 6859
EPS = 1e-5
BB = B // N_CORES        # 2
NTOT = B * NV2
FP1 = 44 * 44            # per-u plane, stored as 4 quadrants of 22x22
QP = 22 * 22             # 484 quadrant plane
SQF = 2 * 37 * 38        # conv1 input quarter-planes (xp, 37 xq, 38 yh)

ZBLK = [(0, 0, 10, 5), (1, 5, 20, 5), (2, 15, 30, 5), (3, 25, 40, 5),
        (4, 35, 50, 5), (5, 45, 60, 5), (6, 55, 64, 4)]
VAR = [(10, 5, 5), (15, 5, 0), (9, 4, 0)]   # (nzr, Zo, kzoff)
KY0 = [0, 2, 4, 6]
XCH = [(0, 15), (15, 30), (30, 34)]          # conv1 x chunks -> free 510/510/136
PADN = 8 * 1715          # 13720: BB*NV2 (13718) padded to 8 chunks
CV = PADN // 8           # 1715


# ---------------- host-side weight prep ----------------

def _build_w1t(W1, basis1):
    """conv1 weights; psum col order (i, uhalf, zor, u4)."""
    K1 = np.einsum('uvb,bixyz->uivxyz', W1, basis1[:, :, 0]).reshape(24, 4, K, K, K)
    out = np.zeros((3, 28, 120, 120), np.float32)
    for vi, (nzr, Zo, kzoff) in enumerate(VAR):
        zr = np.arange(nzr)[:, None]
        zor = np.arange(Zo)[None, :]
        kz = zr - 2 * zor + kzoff
        mask = (kz >= 0) & (kz < 7)
        kzc = np.clip(kz, 0, 6)
        for kx in range(7):
            for yg in range(4):
                g = kx * 4 + yg
                nsh = 2 if yg < 3 else 1
                for s2 in range(nsh):
                    ky = KY0[yg] + s2
                    vals = K1[:, :, kx, ky, :][:, :, kzc] * mask      # [24,4,nzr,Zo]
                    m = vals.transpose(2, 1, 0, 3)                    # [zr,ci,co,zor]
                    m = m.reshape(nzr, 4, 8, 3, Zo)                   # co=(u,i)
                    # cols: (i, uh, zor, u4)
                    m = m.reshape(nzr, 4, 2, 4, 3, Zo)                # (u -> uh,u4)
                    m = m.transpose(0, 1, 4, 2, 5, 3)                 # [zr,ci,i,uh,zor,u4]
                    m = m.reshape(4 * nzr, 3 * Zo * 8)
                    out[vi, g, s2 * 4 * nzr:(s2 + 1) * 4 * nzr, :3 * Zo * 8] = m
    # device layout: [row(120), (vi*28+g)*120 + col]
    return np.ascontiguousarray(
        out.transpose(2, 0, 1, 3).reshape(120, 3 * 28 * 120)).astype(BF16)


def _build_w2t(basis2a, basis2b):
    zeta = np.arange(D1)[:, None]
    zo2 = np.arange(D2)[None, :]
    kz = zeta - 2 * zo2 + 5
    mask = (kz >= 0) & (kz < 7)
    kzc = np.clip(kz, 0, 6)
    W = np.zeros((3, 49, 128, 64), np.float32)
    for fam in range(3):
        for t in range(49):
            kx, ky = divmod(t, 7)
            for i in range(3):
                if fam == 0:
                    prof = basis2a[:, 0, i, kx, ky, :]
                elif fam == 1:
                    prof = basis2b[:, 0, i * 3 + i, kx, ky, :]
                else:
                    p = (i + 1) % 3
                    prof = basis2b[:, 0, i * 3 + p, kx, ky, :] + \
                        basis2b[:, 0, p * 3 + i, kx, ky, :]
                for b in range(NB):
                    vals = prof[b][kzc] * mask
                    W[fam, t, i * D1:(i + 1) * D1, b * D2:(b + 1) * D2] = vals
    W = W.reshape(147, 128, 64)
    return np.ascontiguousarray(
        W.transpose(1, 0, 2).reshape(128, 147 * 64)).astype(BF16)


def _build_wmix(W2a, W2b):
    M = np.zeros((48, 16), np.float32)
    for famM, W2 in [(0, W2a), (1, W2b)]:
        for u in range(VEC):
            for b in range(NB):
                M[famM * 24 + u * 3 + b, :] = W2[:, u, b]
    return M.astype(BF16)


def _prep_s(s_core):
    """[BB,4,64,64,64] -> 7 arrays [BB, 2*4*win, 2*37*38] bf16.
    row = s2*(4*win) + zr*4 + ci ; s2 rows hold the even(0)/odd(1) y
    half-plane, with free split by x parity: free = xp*37*38 + xq*38 + yh.
    Axes: x=D(pad 74), y=H(pad 76), z=W."""
    sp = np.zeros((BB, 4, 74, 76, 64), np.float32)
    sp[:, :, 5:69, 5:69, :] = s_core
    out = []
    for zb, wlo, whi, Zo in ZBLK:
        win = whi - wlo
        sl = sp[:, :, :, :, wlo:whi]                            # [BB,4,74,76,win]
        rows = []
        for s2 in range(2):
            a = sl[:, :, :, s2::2, :][:, :, :, :38, :]
            if a.shape[3] < 38:
                a = np.pad(a, ((0, 0), (0, 0), (0, 0),
                               (0, 38 - a.shape[3]), (0, 0)))
            # x-parity split: [BB,4,xp(2),37,38,win]
            a = np.stack([a[:, :, 0::2], a[:, :, 1::2]], axis=2)
            a = a.transpose(0, 5, 1, 2, 3, 4)                   # [BB,win,4,2,37,38]
            rows.append(a.reshape(BB, win * 4, SQF))
        out.append(np.ascontiguousarray(
            np.concatenate(rows, axis=1)).astype(BF16))
    return out


# ---------------- device program ----------------

def _build_program(n_cores):
    import concourse.bacc as bacc
    import concourse.mybir as mybir
    import concourse.tile as tile

    F32 = mybir.dt.float32
    BF = mybir.dt.bfloat16
    AF = mybir.ActivationFunctionType

    nc = bacc.Bacc("TRN2", target_bir_lowering=False, debug=False,
                   enable_asserts=True, num_devices=n_cores,
                   num_swdge_queues=4)

    sq_d = [nc.dram_tensor(f"sq{zb}", [BB, 8 * (whi - wlo), SQF], BF,
                           kind="ExternalInput").ap()
            for zb, wlo, whi, Zo in ZBLK]
    w1t_d = nc.dram_tensor("w1t", [120, 3 * 28 * 120], BF, kind="ExternalInput").ap()
    w2t_d = nc.dram_tensor("w2t", [128, 147 * 64], BF, kind="ExternalInput").ap()
    wmix_d = nc.dram_tensor("wmix", [48, 16], BF, kind="ExternalInput").ap()
    gvec_d = nc.dram_tensor("gvec", [16, 2], F32, kind="ExternalInput").ap()
    yout_d = nc.dram_tensor("yout", [16, PADN], F32, kind="ExternalOutput").ap()

    with tile.TileContext(nc) as tc:
        with tc.tile_pool(name="wpool", bufs=1) as wpool, \
             tc.tile_pool(name="big", bufs=1) as big, \
             tc.tile_pool(name="sqp", bufs=6) as sqp, \
             tc.tile_pool(name="tp", bufs=2) as tpp, \
             tc.tile_pool(name="d2s", bufs=2) as d2sp, \
             tc.tile_pool(name="bn", bufs=1) as bnp, \
             tc.tile_pool(name="ps", bufs=2, space="PSUM") as psp, \
             tc.tile_pool(name="dram", bufs=1, space="DRAM") as dramp:

            w1t = wpool.tile([120, 3 * 28 * 120], BF, tag="w1t")
            w2t = wpool.tile([128, 147 * 64], BF, tag="w2t")
            wmix = wpool.tile([48, 16], BF, tag="wmix")
            gvec = wpool.tile([16, 2], F32, tag="gvec")

            # u-half-split v tiles: [102, 4u, quadrant(2,2), 22, 22]
            vhalves = []
            for nm in ("vml", "vmh", "vpl", "vph"):
                t = big.tile([102, 4 * FP1], BF, tag=nm, name=nm)
                vhalves.append(t)
            vml, vmh, vpl, vph = vhalves
            # memsets split across engines (pads must be zero)
            nc.vector.memset(vml[:], 0.0)
            nc.vector.memset(vpl[:], 0.0)
            nc.gpsimd.memset(vmh[:], 0.0)
            nc.gpsimd.memset(vph[:], 0.0)

            m_in = big.tile([48, NV2], BF, tag="min")
            yfin = big.tile([128, CV], F32, tag="yfin")
            nc.vector.memset(yfin[:, CV - 2:CV], 0.0)   # pad tail
            sqscr = big.tile([128, 512], F32, tag="sqscr")  # square scratch

            vstgs = [big.tile([120, FP1], BF, tag=f"vstg{i}", name=f"vstg{i}")
                     for i in range(7)]
            for i, v in enumerate(vstgs):
                (nc.gpsimd if i % 2 == 0 else nc.vector).memset(v[:], 0.0)

            bn_in = dramp.tile([16, 2], F32, tag="bnin")
            bn_out = dramp.tile([16, 2], F32, tag="bnout")

            # views: [102, u4, xp, yp, 22, 22]
            def qview(t):
                return t[:].rearrange("p (u a b x y) -> p u a b x y",
                                      u=4, a=2, b=2, x=22)
            vmlq, vmhq, vplq, vphq = [qview(t) for t in vhalves]

            # initial loads; ring discipline:
            #  sync   = w1t slabs + lo-gathers (+ a few sq)
            #  scalar = sq zb0/2/4/6 + sq bb1 prefetch + m_in drains + mix
            #  gpsimd = sq zb1/3/5 + w2t + hi-gathers
            nc.sync.dma_start(w1t[:, 0:14 * 120], w1t_d[:, 0:14 * 120])
            nc.scalar.dma_start(w1t[:, 14 * 120:28 * 120],
                                w1t_d[:, 14 * 120:28 * 120])
            for vi0 in (1, 2):
                nc.sync.dma_start(w1t[:, vi0 * 28 * 120:(vi0 + 1) * 28 * 120],
                                  w1t_d[:, vi0 * 28 * 120:(vi0 + 1) * 28 * 120])
            sq_tiles = {}
            ld_rings = {0: nc.scalar, 1: nc.gpsimd, 2: nc.scalar, 3: nc.gpsimd,
                        4: nc.scalar, 5: nc.gpsimd, 6: nc.scalar}
            for zb, wlo, whi, Zo in ZBLK:
                win = whi - wlo
                sqt = sqp.tile([8 * win, SQF], BF, tag="sqz", name="sqt")
                ld_rings[zb].dma_start(sqt[:], sq_d[zb][0])
                sq_tiles[(0, zb)] = sqt
            nc.gpsimd.dma_start(w2t[:], w2t_d[:])
            nc.scalar.dma_start(wmix[:], wmix_d[:])
            nc.scalar.dma_start(gvec[:], gvec_d[:])
            rings = [nc.sync, nc.scalar, nc.gpsimd]

            def emit_conv1_zb(bb, zbi):
                zb, wlo, whi, Zo = ZBLK[zbi]
                win = whi - wlo
                vi = 0 if zb == 0 else (2 if zb == 6 else 1)
                ncols = 24 * Zo
                sqt = sq_tiles[(bb, zb)]
                sqv = sqt[0:8 * win, :].rearrange("p (e x y) -> p e x y",
                                                  e=2, y=38)
                vstg = vstgs[zbi]
                vsq = vstg[:].rearrange("p (a b x y) -> p a b x y",
                                        a=2, b=2, x=22)
                rows = 8 * win
                for cc, (clo, chi) in enumerate(XCH):
                    cx = chi - clo
                    pc = psp.tile([128, 512], F32, tag="pc", bufs=2)
                    for kx in range(7):
                        for yg in range(4):
                            g = kx * 4 + yg
                            lhs = w1t[0:rows, (vi * 28 + g) * 120:
                                      (vi * 28 + g) * 120 + ncols]
                            xq0 = clo + kx // 2
                            rhs = sqv[0:rows, kx % 2, xq0:xq0 + cx,
                                      yg:yg + 34]
                            nc.tensor.matmul(pc[0:ncols, 0:cx * 34], lhs, rhs,
                                             start=(g == 0), stop=(g == 27))
                    # psum -> vstg quadrant copies (4 per chunk)
                    pcv = pc[0:ncols, 0:cx * 34].rearrange(
                        "p (x y) -> p x y", y=34)
                    x0g = 5 + clo                    # global x of chunk col 0
                    for a in range(2):
                        dx0 = (a - x0g) % 2          # first dx with parity a
                        if dx0 >= cx:
                            continue
                        nxa = (cx - dx0 + 1) // 2
                        xq0d = (x0g + dx0 - a) // 2
                        for bpar in range(2):
                            dy0 = (bpar - 5) % 2     # first dy: y=5+dy par b
                            yq0d = (5 + dy0 - bpar) // 2
                            nc.vector.tensor_copy(
                                vsq[0:ncols, a, bpar,
                                    xq0d:xq0d + nxa, yq0d:yq0d + 17],
                                pcv[:, dx0::2, dy0::2])

            def emit_prefetch(zbi):
                zb, wlo, whi, Zo = ZBLK[zbi]
                win = whi - wlo
                sqt1 = sqp.tile([8 * win, SQF], BF, tag="sqz", name="sqt1")
                nc.scalar.dma_start(sqt1[:], sq_d[zb][1])
                sq_tiles[(1, zb)] = sqt1

            def emit_gather(zbi, uh):
                # vstg rows (i, uh, zor, u4) -> v half tiles.  lo halves on
                # sync, hi halves on gpsimd (separate rings so a WAR-blocked
                # hi gather never clogs the lo ring).  Emission point defines
                # program-order semantics: only emit a half's gather once all
                # prior readers of that half have been emitted.
                zb, wlo, whi, Zo = ZBLK[zbi]
                vstg = vstgs[zbi]
                zsp = [(0, Zo)] if zbi < 5 else [(0, 3), (3, Zo)]
                for i in range(3):
                    c = (i + 2) % 3
                    base = (i * 2 + uh) * 4 * Zo
                    vmq = vmlq if uh == 0 else vmhq
                    vpq = vplq if uh == 0 else vphq
                    ring = nc.sync if uh == 0 else nc.gpsimd
                    for z0, z1 in zsp:
                        src = vstg[base + z0 * 4: base + z1 * 4, :]
                        dsts = [vmq[i * D1 + 5 * zb + z0:
                                    i * D1 + 5 * zb + z1, :, :, :, :, :],
                                vpq[c * D1 + 5 * zb + z0:
                                    c * D1 + 5 * zb + z1, :, :, :, :, :]]
                        for dst in dsts:
                            ring.dma_start(dst, src)

            def emit_conv2_u(bb, u):
                vh = vhalves[0] if u < 4 else vhalves[1]
                vp = vhalves[2] if u < 4 else vhalves[3]
                u4 = u % 4
                vmu = vh[:].rearrange("p (u f) -> p u f", u=4)[:, u4, :]
                vpu = vp[:].rearrange("p (u f) -> p u f", u=4)[:, u4, :]
                t1u = tpp.tile([102, FP1], BF, tag="t1u")
                t2u = tpp.tile([102, FP1], BF, tag="t2u")
                nc.vector.tensor_mul(t1u[:], vmu, vmu)
                nc.vector.tensor_mul(t2u[:], vmu, vpu)
                pd2a = psp.tile([64, 512], F32, tag="pd2a", bufs=2)
                pd2b = psp.tile([64, 512], F32, tag="pd2b", bufs=2)
                pav = pd2a[0:57, 0:XY2].rearrange("p (x y) -> p x y", y=D2)
                pbv = pd2b[0:57, 0:XY2].rearrange("p (x y) -> p x y", y=D2)
                vmuq = vmu.rearrange("p (a b x y) -> p a b x y",
                                     a=2, b=2, x=22)
                t1q = t1u[:].rearrange("p (a b x y) -> p a b x y",
                                       a=2, b=2, x=22)
                t2q = t2u[:].rearrange("p (a b x y) -> p a b x y",
                                       a=2, b=2, x=22)
                for fam, rq, pv in ((0, vmuq, pav), (1, t1q, pbv),
                                    (2, t2q, pbv)):
                    for t in range(49):
                        kx, ky = divmod(t, 7)
                        rhs = rq[:, kx % 2, ky % 2,
                                 kx // 2:kx // 2 + 19,
                                 ky // 2:ky // 2 + 19]
                        lhs = w2t[0:102,
                                  (fam * 49 + t) * 64:(fam * 49 + t) * 64 + 57]
                        nc.tensor.matmul(pv[:, :, :], lhs, rhs,
                                         start=(t == 0 and fam != 2),
                                         stop=(t == 48 and fam != 1))
                for famM, psrc in ((0, pd2a), (1, pd2b)):
                    stg = d2sp.tile([57, XY2], BF, tag=f"stg{famM}")
                    nc.vector.tensor_copy(stg[:], psrc[0:57, 0:XY2])
                    # SBUF->SBUF: [57=(b,zo), 361] -> m_in [3 rows, 6859]
                    nc.scalar.dma_start(
                        m_in[famM * 24 + u * 3: famM * 24 + u * 3 + 3, :],
                        stg[:])

            def emit_mix(bb):
                nchunks = (NV2 + 511) // 512
                for ch in range(nchunks):
                    c0 = ch * 512
                    cn = min(512, NV2 - c0)
                    pm = psp.tile([16, 512], F32, tag="pm", bufs=2)
                    nc.tensor.matmul(pm[0:16, 0:cn], wmix[:], m_in[:, c0:c0 + cn],
                                     start=True, stop=True)
                    ymix = d2sp.tile([16, 512], F32, tag="ymix", bufs=3)
                    if ch % 2 == 0:
                        nc.vector.tensor_copy(ymix[0:16, 0:cn], pm[0:16, 0:cn])
                    else:
                        nc.scalar.copy(ymix[0:16, 0:cn], pm[0:16, 0:cn])
                    # fold into yfin[j*16+c, v]: global g = bb*NV2 + c0 + k
                    g0 = bb * NV2 + c0
                    g1 = g0 + cn
                    j0, v0 = divmod(g0, CV)
                    j1 = (g1 - 1) // CV
                    ring = (nc.sync, nc.scalar)[ch % 2]
                    if j0 == j1:
                        ring.dma_start(
                            yfin[j0 * 16:(j0 + 1) * 16, v0:v0 + cn],
                            ymix[0:16, 0:cn])
                    else:
                        n1 = CV - v0
                        ring.dma_start(
                            yfin[j0 * 16:(j0 + 1) * 16, v0:CV],
                            ymix[0:16, 0:n1])
                        ring.dma_start(
                            yfin[j1 * 16:(j1 + 1) * 16, 0:cn - n1],
                            ymix[0:16, n1:cn])

            # ---- software-pipelined schedule ----
            # bb0 conv1: compute + both gathers immediately (v tiles fresh)
            for zbi in range(7):
                emit_conv1_zb(0, zbi)
                emit_gather(zbi, 0)
                emit_gather(zbi, 1)
                if zbi >= 2:
                    emit_prefetch(zbi - 2)
            for zbi in (5, 6):
                emit_prefetch(zbi)
            # conv2-bb0 u0..3 read the lo halves; after u3 all of bb1's
            # conv1 runs (private vstg buffers), each z-block's lo gather
            # emitted immediately (only hi planes still being read by
            # u4..7).  Hi gathers wait until after u7 and run on gpsimd
            # under conv2-bb1's u0..3.
            for u in range(4):
                emit_conv2_u(0, u)
            for zbi in range(7):
                emit_conv1_zb(1, zbi)
                emit_gather(zbi, 0)
            for u in range(4, VEC):
                emit_conv2_u(0, u)
            for zbi in range(7):
                emit_gather(zbi, 1)
            emit_mix(0)
            for u in range(VEC):
                emit_conv2_u(1, u)
            emit_mix(1)

            # ---------------- stats + batchnorm all-reduce + finalize ------
            s128 = bnp.tile([128, 8], F32, tag="s128")
            nc.vector.reduce_sum(s128[:, 6:7], yfin[:], axis=mybir.AxisListType.X)
            SC4 = (CV + 3) // 4
            for c in range(4):
                v0 = c * SC4
                v1 = min(CV, v0 + SC4)
                nc.scalar.activation(sqscr[:, 0:v1 - v0], yfin[:, v0:v1],
                                     AF.Square, accum_out=s128[:, 1 + c:2 + c])
            nc.vector.reduce_sum(s128[:, 7:8], s128[:, 1:5],
                                 axis=mybir.AxisListType.X)
            # fold 128 -> 16: [128,2] -> [16, 16] (j-major pairs)
            fold = bnp.tile([16, 16], F32, tag="fold")
            for j in range(8):
                rings[j % 2].dma_start(fold[:, j * 2:(j + 1) * 2],
                                       s128[j * 16:(j + 1) * 16, 6:8])
            bnv = bnp.tile([16, 2], F32, tag="bnv")
            fv2 = fold[:].rearrange("p (j s) -> p s j", s=2)
            nc.vector.reduce_sum(bnv[:, 0:1], fv2[:, 0, :], axis=mybir.AxisListType.X)
            nc.vector.reduce_sum(bnv[:, 1:2], fv2[:, 1, :], axis=mybir.AxisListType.X)
            nc.sync.dma_start(bn_in[:], bnv[:])
            nc.gpsimd.collective_compute(
                "AllReduce", mybir.AluOpType.add,
                replica_groups=[list(range(n_cores))],
                ins=[bn_in[:].opt()], outs=[bn_out[:].opt()])
            bnr = bnp.tile([16, 2], F32, tag="bnr")
            nc.sync.dma_start(bnr[:], bn_out[:])
            w = bnp.tile([16, 8], F32, tag="bnw")
            invN = 1.0 / float(NTOT)
            nc.vector.tensor_scalar_mul(w[:, 0:1], bnr[:, 0:1], invN)   # mean
            nc.vector.tensor_scalar_mul(w[:, 1:2], bnr[:, 1:2], invN)   # E[x^2]
            nc.vector.tensor_mul(w[:, 2:3], w[:, 0:1], w[:, 0:1])       # mean^2
            nc.vector.tensor_sub(w[:, 3:4], w[:, 1:2], w[:, 2:3])       # var
            nc.vector.tensor_scalar_add(w[:, 4:5], w[:, 3:4], EPS)
            nc.vector.reciprocal(w[:, 5:6], w[:, 4:5])
            nc.scalar.sqrt(w[:, 6:7], w[:, 5:6])                        # rstd
            sc = bnp.tile([16, 2], F32, tag="bnsc")
            nc.vector.tensor_mul(sc[:, 0:1], gvec[:, 0:1], w[:, 6:7])   # scale
            nc.vector.tensor_mul(w[:, 7:8], w[:, 0:1], sc[:, 0:1])      # mean*sc
            nc.vector.tensor_sub(sc[:, 1:2], gvec[:, 1:2], w[:, 7:8])   # shift
            # broadcast to 128 partitions: 8 parallel DMAs across rings
            sc128 = bnp.tile([128, 2], F32, tag="sc128")
            for j in range(8):
                rings[j % 2].dma_start(sc128[j * 16:(j + 1) * 16, :], sc[:])
            # final: scale/shift + relu in place, write out (4 chunks,
            # store overlaps the next chunk's activation)
            yov = yout_d[:].rearrange("c (j v) -> j c v", j=8)
            NC4 = (CV + 3) // 4
            for k in range(4):
                v0 = k * NC4
                v1 = min(CV, v0 + NC4)
                nc.scalar.activation(yfin[:, v0:v1], yfin[:, v0:v1], AF.Relu,
                                     bias=sc128[:, 1:2], scale=sc128[:, 0:1])
                rings[k % 2].dma_start(yov[:, :, v0:v1], yfin[:, v0:v1])

    nc.compile()
    return nc


_CACHE = {}


def _get_program(n_cores):
    if n_cores not in _CACHE:
        _CACHE[n_cores] = _build_program(n_cores)
    return _CACHE[n_cores]


def _make_in_maps(inputs):
    s = np.asarray(inputs['s'], np.float32)
    w1t = _build_w1t(np.asarray(inputs['W1'], np.float32),
                     np.asarray(inputs['basis1'], np.float32))
    w2t = _build_w2t(np.asarray(inputs['basis2a'], np.float32),
                     np.asarray(inputs['basis2b'], np.float32))
    wmix = _build_wmix(np.asarray(inputs['W2a'], np.float32),
                       np.asarray(inputs['W2b'], np.float32))
    gvec = np.stack([np.asarray(inputs['gamma'], np.float32),
                     np.asarray(inputs['beta'], np.float32)
                     + np.asarray(inputs['bias'], np.float32)], axis=1)
    in_maps = []
    for c in range(N_CORES):
        sqs = _prep_s(s[BB * c: BB * (c + 1)])
        m = {f"sq{zb}": sqs[zb] for zb in range(7)}
        m.update({"w1t": w1t, "w2t": w2t, "wmix": wmix,
                  "gvec": np.ascontiguousarray(gvec)})
        in_maps.append(m)
    return in_maps


def _assemble(results):
    out = np.zeros((B, 16, D2, D2, D2), np.float32)
    for c in range(N_CORES):
        yo = results[c]["yout"]           # [16, 13720] (padded)
        for bb in range(BB):
            yb = yo[:, bb * NV2:(bb + 1) * NV2].reshape(16, D2, D2, D2)
            out[BB * c + bb] = yb.transpose(0, 2, 3, 1)  # (z,x,y)->(x,y,z)
    return out


def _run(inputs, trace=False, trace_kwargs=None):
    from concourse import bass_utils
    nc = _get_program(N_CORES)
    in_maps = _make_in_maps(inputs)
    res = bass_utils.run_bass_kernel_spmd(
        nc, in_maps, core_ids=list(range(N_CORES)), trace=trace,
        **(trace_kwargs or {}))
    return _assemble(res.results), res


def kernel(**inputs):
    out, _ = _run(inputs, trace=False)
    return out
